# revision 40
# baseline (speedup 1.0000x reference)
"""Trainium2 Bass kernel for batched multi-head attention.

Problem: q, k, v: [B=4, H=16, D=64, N=2048] fp32, layout (b, h, d, n).
    sim  = einsum('bhdi,bhdj->bhij', q * D**-0.5, k)
    attn = softmax(sim, axis=-1)
    out  = einsum('bhij,bhdj->bhdi', attn, v)

Sharding: the 64 (b, h) pairs split across 8 NeuronCores, 8 heads per core.
Each core runs an identical Bass program on its own slice; no collectives.

Per-core algorithm (flash-style, S^T layout so no P transposes are needed):
  per head pair: V^T chunks via PE transpose (both heads in one [128,128] block)
  per head, per i-tile (512 query cols):
    per group of key chunks (3,3,3,3,2,2 chunks of 128 keys):
      S^T[j, i] = K_chunk^T Q_tile     (fp32r matmuls -> 3-bank PSUM slot)
      expS = exp(0.125 * S^T)          (one ACT instr over the whole slot)
      O_acc[d|sum, i] += [V^T | 1]^T expS   (fp32r matmuls into a dedicated
                                        1-bank PSUM accumulator, all 16 chunks)
    out[d, i] = O_acc[d] * (1 / O_acc[64])  (DVE recip + DRAM-bounce broadcast)

PSUM: 2 x 3-bank S slots + 2 x 1-bank O accumulators = 8 banks.
S slots are released by ACT alone, so the steady-state cycle is
exp(g-2) -> S(g) -> exp(g): ACT (the roofline engine, 8 * 2048^2 exp elems /
128 lanes / 1.2 GHz ~= 220 us/core) runs back-to-back. Emission is
software-pipelined (S of group g+1 before O of group g). The next pair's
V^T transposes ride in the spare bank of the 2-chunk S slots (pair 0 stages
through the then-idle O accumulator banks), so pair switches cost no slot
turns.
"""

import numpy as np

import concourse.bass as bass
import concourse.mybir as mybir
import concourse.tile as tile
from concourse import bacc
from concourse.bass_utils import run_bass_kernel_spmd
from concourse.masks import make_identity

B, H, D, N = 4, 16, 64, 2048
NCORES = 8
HPC = (B * H) // NCORES  # heads per core = 8
NPAIRS = HPC // 2        # head pairs per core = 4
ITILE = 512              # query columns per i-tile (PSUM bank = 512 fp32)
NIT = N // ITILE         # 4 i-tiles
JC = 128                 # key chunk (matmul M / partition dim)
NCH = N // JC            # 16 chunks
SCALE = float(D) ** -0.5

# chunk groups per i-tile: (start, count). The two 2-chunk groups leave a
# spare 512-fp32 bank in the 3-bank S slot for staging V^T transposes; they
# are positioned so the transpose+copy perturbation of a slot's release is
# followed by a cheap (2-chunk) S refill two groups later.
GROUPS_PER_IT = [(0, 3), (3, 3), (6, 3), (9, 3), (12, 2), (14, 2)]
NGT = len(GROUPS_PER_IT)

F32 = mybir.dt.float32
F32R = mybir.dt.float32r

_CACHE = {}


def build_bass():
    nc = bacc.Bacc("TRN2", target_bir_lowering=False)
    qkv_h = nc.dram_tensor("qkv", [NPAIRS, 3, 128, N], F32, kind="ExternalInput")
    o_h = nc.dram_tensor("out", [HPC, D, N], F32, kind="ExternalOutput")

    qkv_d = qkv_h[:, :, :, :].rearrange("p t a n -> p a t n")  # [NPAIRS, 128, 3, N]
    o_d = o_h[:, :, :].rearrange("(p two) d n -> p (two d) n", two=2)

    with tile.TileContext(nc) as tc:
        with (
            tc.tile_pool(name="consts", bufs=1) as consts,
            tc.tile_pool(name="pairs", bufs=2) as pairs,
            tc.tile_pool(name="spsum", bufs=2, space="PSUM") as spsum,
            tc.tile_pool(name="opsum", bufs=2, space="PSUM") as opsum,
            tc.tile_pool(name="expp", bufs=3) as expp,
            tc.tile_pool(name="vtp", bufs=2) as vtp,
            tc.tile_pool(name="oaccp", bufs=2) as oaccp,
            tc.tile_pool(name="outp", bufs=3) as outp,
            tc.tile_pool(name="dramp", bufs=3, space="DRAM") as dramp,
        ):
            ident = consts.tile([128, 128], F32, tag="ident")
            make_identity(nc, ident)
            ones_dram = nc.inline_tensor(np.ones([1, 1], np.float32), name="ones_const")

            pair_ctx: list[dict | None] = [None] * NPAIRS

            def emit_pair_dma(p):
                q3 = pairs.tile([128, 3, N], F32R, tag="q3", name=f"q3_{p}")
                # split q+k from v so the S pipeline starts before v lands;
                # pair 0 additionally peels the first S group's operands so
                # the ramp isn't gated on the full 2 MB q+k transfer
                src = qkv_d[p].bitcast(F32R)
                if p == 0:
                    half = N // 2
                    nc.sync.dma_start(out=q3[:, 0, 0:ITILE], in_=src[:, 0, 0:ITILE])
                    nc.sync.dma_start(out=q3[:, 1, 0:half], in_=src[:, 1, 0:half])
                    nc.sync.dma_start(out=q3[:, 1, half:N], in_=src[:, 1, half:N])
                    for quarter in range(4):
                        n0 = quarter * (N // 4)
                        nc.sync.dma_start(
                            out=q3[:, 2, n0 : n0 + N // 4],
                            in_=src[:, 2, n0 : n0 + N // 4],
                        )
                    nc.sync.dma_start(out=q3[:, 0, ITILE:N], in_=src[:, 0, ITILE:N])
                else:
                    nc.sync.dma_start(out=q3[:, 0:2, :], in_=src[:, 0:2, :])
                    nc.sync.dma_start(out=q3[:, 2:3, :], in_=src[:, 2:3, :])
                vt_sb = vtp.tile([128, NCH, 2, D + 1], F32R, tag="vt", name=f"vt_{p}")
                ones_bcast = bass.AP(
                    tensor=ones_dram, offset=0, ap=[[0, 128], [0, 2 * NCH]]
                )
                nc.sync.dma_start(
                    out=vt_sb.rearrange("a b c d -> a (b c) d")[:, :, D : D + 1],
                    in_=ones_bcast.bitcast(F32R),
                )
                pair_ctx[p] = {"q3": q3, "vt": vt_sb}

            def emit_transposes(p, c0, cn, ps_region):
                """Transpose V chunks [c0, c0+cn) of pair p into ps_region
                ([128, cn*128] PSUM), then copy into vt_sb."""
                cx = pair_ctx[p]
                q3 = cx["q3"]
                for c in range(c0, c0 + cn):
                    nc.tensor.transpose(
                        out=ps_region[:, (c - c0) * 128 : (c - c0 + 1) * 128],
                        in_=q3[:, 2, c * 128 : (c + 1) * 128].bitcast(F32),
                        identity=ident,
                    )
                nc.vector.tensor_copy(
                    out=cx["vt"][:, c0 : c0 + cn, :, 0:D],
                    in_=ps_region[:, 0 : cn * 128].rearrange(
                        "a (c h d) -> a c h d", h=2, d=D
                    ),
                )

            def emit_pair0_batch(b):
                # o-slots are idle at ramp: stage pair 0's V^T through them
                vt_ps = opsum.tile([128, ITILE], F32, tag="o", name=f"vtps0_{b}")
                emit_transposes(0, 4 * b, 4, vt_ps)

            groups = [
                (p, sub, it, gi)
                for p in range(NPAIRS)
                for sub in range(2)
                for it in range(NIT)
                for gi in range(NGT)
            ]
            slots: dict[int, bass.AP] = {}
            exs: dict[int, bass.AP] = {}
            o_pss: dict[tuple, bass.AP] = {}

            # Each i-tile of the second head stages 4 chunks of pair p+1's V^T
            # through the o-slot that idles between norm(it-1) release and
            # accum(it+1) alloc. Only sub 1 rides, so the next pair's v DMA
            # (issued a full pair ahead) has certainly landed and the in-order
            # PE queue never blocks on it.
            RIDE = {(1, it): it for it in range(NIT)}

            def emit_S(i):
                p, sub, it, gi = groups[i]
                if sub == 0 and it == 0 and gi == 0:
                    if p == 0:
                        emit_pair_dma(0)
                    if p + 1 < NPAIRS:
                        emit_pair_dma(p + 1)
                cx = pair_ctx[p]
                q3 = cx["q3"]
                hb = sub * D
                i0 = it * ITILE
                c0, cn = GROUPS_PER_IT[gi]
                s_slot = spsum.tile([128, 3 * ITILE], F32, tag="s", name=f"s_{i}")
                for kk in range(cn):
                    c = c0 + kk
                    nc.tensor.matmul(
                        out=s_slot[:, kk * ITILE : (kk + 1) * ITILE],
                        lhsT=q3[hb : hb + D, 1, c * JC : (c + 1) * JC],
                        rhs=q3[hb : hb + D, 0, i0 : i0 + ITILE],
                        start=True,
                        stop=True,
                    )
                slots[i] = s_slot

            def emit_X(i):
                p, sub, it, gi = groups[i]
                c0, cn = GROUPS_PER_IT[gi]
                ex = expp.tile([128, cn * ITILE], F32R, tag="exp", name=f"x_{i}")
                nc.scalar.activation(
                    out=ex,
                    in_=slots[i][:, 0 : cn * ITILE],
                    func=mybir.ActivationFunctionType.Exp,
                    scale=SCALE,
                )
                exs[i] = ex

            def emit_O(i):
                p, sub, it, gi = groups[i]
                cx = pair_ctx[p]
                s_slot, ex = slots.pop(i), exs.pop(i)
                c0, cn = GROUPS_PER_IT[gi]
                if gi == 0:
                    o_ps = opsum.tile([128, ITILE], F32, tag="o", name=f"ops_{i}")
                    o_pss[(p, sub, it)] = o_ps
                else:
                    o_ps = o_pss[(p, sub, it)]
                for kk in range(cn):
                    c = c0 + kk
                    nc.tensor.matmul(
                        out=o_ps[0 : D + 1, :],
                        lhsT=cx["vt"][:, c, sub, :],
                        rhs=ex[:, kk * ITILE : (kk + 1) * ITILE],
                        start=(c == 0),
                        stop=(c == NCH - 1),
                    )
                if gi == 1 and p + 1 < NPAIRS and (sub, it) in RIDE:
                    batch = RIDE[(sub, it)]
                    vt_ps = opsum.tile([128, ITILE], F32, tag="o", name=f"vtr_{i}")
                    emit_transposes(p + 1, 4 * batch, 4, vt_ps)
                if gi == NGT - 1:
                    emit_norm(p, sub, it)

            def emit_norm(p, sub, it):
                hb = sub * D
                i0 = it * ITILE
                o_ps = o_pss.pop((p, sub, it))
                rec = oaccp.tile([1, ITILE], F32, tag="rec", name=f"r_{p}_{sub}_{it}")
                nc.vector.reciprocal(out=rec, in_=o_ps[D : D + 1, :])
                recb = oaccp.tile([D, ITILE], F32, tag="recb", name=f"rb_{p}_{sub}_{it}")
                nc.gpsimd.partition_broadcast(recb, rec)
                out_sb = outp.tile([D, ITILE], F32, tag="out_sb", name=f"ot_{p}_{sub}_{it}")
                nc.vector.tensor_mul(out=out_sb, in0=o_ps[0:D, :], in1=recb)
                nc.sync.dma_start(
                    out=o_d[p][hb : hb + D, i0 : i0 + ITILE], in_=out_sb
                )

            # software-pipelined emission: S(i+1) lands before O(i)
            emit_S(0)
            for i in range(len(groups)):
                emit_X(i)
                if i + 1 < len(groups):
                    emit_S(i + 1)
                if i < 4:
                    emit_pair0_batch(i)
                emit_O(i)

    nc.compile()
    return nc


def kernel(q: np.ndarray, k: np.ndarray, v: np.ndarray) -> np.ndarray:
    if "nc" not in _CACHE:
        _CACHE["nc"] = build_bass()
    nc = _CACHE["nc"]

    in_maps = [{"qkv": pack_core_inputs(q, k, v, c)} for c in range(NCORES)]
    res = run_bass_kernel_spmd(nc, in_maps, core_ids=list(range(NCORES)))
    out = np.concatenate([res.results[c]["out"] for c in range(NCORES)], axis=0)
    return out.reshape(B, H, D, N).astype(np.float32)


def pack_core_inputs(q, k, v, c):
    """[NPAIRS, 3, 128, N] fp32: two heads stacked per pair, q/k/v interleaved."""
    qr = q.reshape(B * H, D, N)[c * HPC : (c + 1) * HPC].reshape(NPAIRS, 128, N)
    kr = k.reshape(B * H, D, N)[c * HPC : (c + 1) * HPC].reshape(NPAIRS, 128, N)
    vr = v.reshape(B * H, D, N)[c * HPC : (c + 1) * HPC].reshape(NPAIRS, 128, N)
    return np.ascontiguousarray(
        np.stack([qr, kr, vr], axis=1).astype(np.float32, copy=False)
    )


if __name__ == "__main__":
    rng = np.random.default_rng(0)
    q = rng.standard_normal((B, H, D, N), dtype=np.float32)
    k = rng.standard_normal((B, H, D, N), dtype=np.float32)
    v = rng.standard_normal((B, H, D, N), dtype=np.float32)
    out = kernel(q, k, v)
    s = np.einsum("hdi,hdj->hij", q.reshape(-1, D, N)[:2] * SCALE, k.reshape(-1, D, N)[:2])
    p = np.exp(s - s.max(-1, keepdims=True))
    p /= p.sum(-1, keepdims=True)
    ref = np.einsum("hij,hdj->hdi", p, v.reshape(-1, D, N)[:2])
    got = out.reshape(-1, D, N)[:2]
    print("rel err (2 heads):", np.linalg.norm(got - ref) / np.linalg.norm(ref))


# revision 49
# speedup vs baseline: 1.0941x; 1.0941x over previous
"""Trainium2 Bass kernel for batched multi-head attention.

Problem: q, k, v: [B=4, H=16, D=64, N=2048] fp32, layout (b, h, d, n).
    sim  = einsum('bhdi,bhdj->bhij', q * D**-0.5, k)
    attn = softmax(sim, axis=-1)
    out  = einsum('bhij,bhdj->bhdi', attn, v)

Sharding: the 64 (b, h) pairs split across 8 NeuronCores, 8 heads per core.
Each core runs an identical Bass program on its own slice; no collectives.

Per-core algorithm (flash-style, S^T layout so no P transposes are needed):
  per head pair: V^T chunks via PE transpose (both heads in one [128,128] block)
  per head, per i-tile (512 query cols):
    per group of key chunks (3,3,3,3,2,2 chunks of 128 keys):
      S^T[j, i] = K_chunk^T Q_tile     (fp32r matmuls -> 3-bank PSUM slot)
      expS = exp(0.125 * S^T)          (one ACT instr over the whole slot)
      O_acc[d|sum, i] += [V^T | 1]^T expS   (fp32r matmuls into a dedicated
                                        1-bank PSUM accumulator, all 16 chunks)
    out[d, i] = O_acc[d] * (1 / O_acc[64])  (DVE recip + DRAM-bounce broadcast)

PSUM: 2 x 3-bank S slots + 2 x 1-bank O accumulators = 8 banks.
S slots are released by ACT alone, so the steady-state cycle is
exp(g-2) -> S(g) -> exp(g): ACT (the roofline engine, 8 * 2048^2 exp elems /
128 lanes / 1.2 GHz ~= 220 us/core) runs back-to-back. Emission is
software-pipelined (S of group g+1 before O of group g). The next pair's
V^T transposes ride in the spare bank of the 2-chunk S slots (pair 0 stages
through the then-idle O accumulator banks), so pair switches cost no slot
turns.
"""

import numpy as np

import concourse.bass as bass
import concourse.mybir as mybir
import concourse.tile as tile
from concourse import bacc
from concourse.bass_utils import run_bass_kernel_spmd
from concourse.masks import make_identity

B, H, D, N = 4, 16, 64, 2048
NCORES = 8
HPC = (B * H) // NCORES  # heads per core = 8
NPAIRS = HPC // 2        # head pairs per core = 4
ITILE = 512              # query columns per i-tile (PSUM bank = 512 fp32)
NIT = N // ITILE         # 4 i-tiles
JC = 128                 # key chunk (matmul M / partition dim)
NCH = N // JC            # 16 chunks
SCALE = float(D) ** -0.5

# chunk groups per i-tile: (start, count). The trailing 2-chunk groups keep
# slot-refill cost below the preceding exp duration at i-tile boundaries.
# The very first i-tile of the kernel ramps with tiny groups so ACT starts
# as soon as the first 64 KB of K has landed.
GROUPS_STEADY = [(0, 3), (3, 3), (6, 3), (9, 3), (12, 2), (14, 2)]
GROUPS_RAMP = [(0, 1), (1, 2), (3, 3), (6, 3), (9, 3), (12, 2), (14, 2)]

F32 = mybir.dt.float32
F32R = mybir.dt.float32r

_CACHE = {}


def build_bass():
    nc = bacc.Bacc("TRN2", target_bir_lowering=False)
    qkv_h = nc.dram_tensor("qkv", [NPAIRS, 3, 128, N], F32, kind="ExternalInput")
    o_h = nc.dram_tensor("out", [HPC, D, N], F32, kind="ExternalOutput")

    qkv_d = qkv_h[:, :, :, :].rearrange("p t a n -> p a t n")  # [NPAIRS, 128, 3, N]
    o_d = o_h[:, :, :].rearrange("(p two) d n -> p (two d) n", two=2)

    with tile.TileContext(nc) as tc:
        with (
            tc.tile_pool(name="consts", bufs=1) as consts,
            tc.tile_pool(name="pairs", bufs=2) as pairs,
            tc.tile_pool(name="spsum", bufs=2, space="PSUM") as spsum,
            tc.tile_pool(name="opsum", bufs=2, space="PSUM") as opsum,
            tc.tile_pool(name="expp", bufs=4) as expp,
            tc.tile_pool(name="vtp", bufs=2) as vtp,
            tc.tile_pool(name="oaccp", bufs=2) as oaccp,
            tc.tile_pool(name="outp", bufs=3) as outp,
            tc.tile_pool(name="dramp", bufs=3, space="DRAM") as dramp,
        ):
            ident = consts.tile([128, 128], F32, tag="ident")
            make_identity(nc, ident)
            ones_dram = nc.inline_tensor(np.ones([1, 1], np.float32), name="ones_const")

            pair_ctx: list[dict | None] = [None] * NPAIRS

            def emit_pair_dma(p):
                q3 = pairs.tile([128, 3, N], F32R, tag="q3", name=f"q3_{p}")
                # split q+k from v so the S pipeline starts before v lands;
                # pair 0 additionally peels the first S group's operands so
                # the ramp isn't gated on the full 2 MB q+k transfer
                src = qkv_d[p].bitcast(F32R)
                if p == 0:
                    nc.sync.dma_start(out=q3[:, 0, 0:ITILE], in_=src[:, 0, 0:ITILE])
                    nc.sync.dma_start(out=q3[:, 1, 0:ITILE], in_=src[:, 1, 0:ITILE])
                    nc.sync.dma_start(out=q3[:, 1, ITILE:N], in_=src[:, 1, ITILE:N])
                    nc.sync.dma_start(out=q3[:, 2, 0 : N // 2], in_=src[:, 2, 0 : N // 2])
                    nc.sync.dma_start(out=q3[:, 2, N // 2 : N], in_=src[:, 2, N // 2 : N])
                    nc.sync.dma_start(out=q3[:, 0, ITILE:N], in_=src[:, 0, ITILE:N])
                else:
                    nc.sync.dma_start(out=q3[:, 0:2, :], in_=src[:, 0:2, :])
                    nc.sync.dma_start(out=q3[:, 2:3, :], in_=src[:, 2:3, :])
                vt_sb = vtp.tile([128, NCH, 2, D + 1], F32R, tag="vt", name=f"vt_{p}")
                ones_bcast = bass.AP(
                    tensor=ones_dram, offset=0, ap=[[0, 128], [0, 2 * NCH]]
                )
                nc.sync.dma_start(
                    out=vt_sb.rearrange("a b c d -> a (b c) d")[:, :, D : D + 1],
                    in_=ones_bcast.bitcast(F32R),
                )
                pair_ctx[p] = {"q3": q3, "vt": vt_sb}

            def emit_transposes(p, c0, cn, ps_region):
                """Transpose V chunks [c0, c0+cn) of pair p into ps_region
                ([128, cn*128] PSUM), then copy into vt_sb."""
                cx = pair_ctx[p]
                q3 = cx["q3"]
                for c in range(c0, c0 + cn):
                    nc.tensor.transpose(
                        out=ps_region[:, (c - c0) * 128 : (c - c0 + 1) * 128],
                        in_=q3[:, 2, c * 128 : (c + 1) * 128].bitcast(F32),
                        identity=ident,
                    )
                nc.vector.tensor_copy(
                    out=cx["vt"][:, c0 : c0 + cn, :, 0:D],
                    in_=ps_region[:, 0 : cn * 128].rearrange(
                        "a (c h d) -> a c h d", h=2, d=D
                    ),
                )

            def emit_pair0_batch(b):
                # o-slots are idle at ramp: stage pair 0's V^T through them
                vt_ps = opsum.tile([128, ITILE], F32, tag="o", name=f"vtps0_{b}")
                emit_transposes(0, 4 * b, 4, vt_ps)

            groups = []
            for p in range(NPAIRS):
                for sub in range(2):
                    for it in range(NIT):
                        gl = GROUPS_STEADY
                        for gi, (c0, cn) in enumerate(gl):
                            groups.append(
                                {
                                    "p": p, "sub": sub, "it": it,
                                    "c0": c0, "cn": cn,
                                    "first": gi == 0,
                                    "last": gi == len(gl) - 1,
                                    # second head's i-tiles stage 4 chunks of
                                    # pair p+1's V^T through the o-slot that
                                    # idles between norm(it-1) and accum(it+1)
                                    "ride": gi == 1 and sub == 1 and p + 1 < NPAIRS,
                                }
                            )
            slots: dict[int, bass.AP] = {}
            exs: dict[int, bass.AP] = {}
            o_pss: dict[tuple, bass.AP] = {}

            def emit_S(i):
                g = groups[i]
                p, sub, it = g["p"], g["sub"], g["it"]
                if sub == 0 and it == 0 and g["first"]:
                    if p == 0:
                        emit_pair_dma(0)
                    if p + 1 < NPAIRS:
                        emit_pair_dma(p + 1)
                cx = pair_ctx[p]
                q3 = cx["q3"]
                hb = sub * D
                i0 = it * ITILE
                c0, cn = g["c0"], g["cn"]
                s_slot = spsum.tile([128, 3 * ITILE], F32, tag="s", name=f"s_{i}")
                for kk in range(cn):
                    c = c0 + kk
                    nc.tensor.matmul(
                        out=s_slot[:, kk * ITILE : (kk + 1) * ITILE],
                        lhsT=q3[hb : hb + D, 1, c * JC : (c + 1) * JC],
                        rhs=q3[hb : hb + D, 0, i0 : i0 + ITILE],
                        start=True,
                        stop=True,
                    )
                slots[i] = s_slot

            def emit_X(i):
                g = groups[i]
                cn = g["cn"]
                ex = expp.tile([128, cn * ITILE], F32R, tag="exp", name=f"x_{i}")
                nc.scalar.activation(
                    out=ex,
                    in_=slots[i][:, 0 : cn * ITILE],
                    func=mybir.ActivationFunctionType.Exp,
                    scale=SCALE,
                )
                exs[i] = ex

            def emit_O(i):
                g = groups[i]
                p, sub, it = g["p"], g["sub"], g["it"]
                cx = pair_ctx[p]
                s_slot, ex = slots.pop(i), exs.pop(i)
                c0, cn = g["c0"], g["cn"]
                if g["first"]:
                    o_ps = opsum.tile([128, ITILE], F32, tag="o", name=f"ops_{i}")
                    o_pss[(p, sub, it)] = o_ps
                else:
                    o_ps = o_pss[(p, sub, it)]
                for kk in range(cn):
                    c = c0 + kk
                    nc.tensor.matmul(
                        out=o_ps[0 : D + 1, :],
                        lhsT=cx["vt"][:, c, sub, :],
                        rhs=ex[:, kk * ITILE : (kk + 1) * ITILE],
                        start=(c == 0),
                        stop=(c == NCH - 1),
                    )
                if g["ride"]:
                    vt_ps = opsum.tile([128, ITILE], F32, tag="o", name=f"vtr_{i}")
                    emit_transposes(p + 1, 4 * it, 4, vt_ps)
                if g["last"]:
                    emit_norm(p, sub, it)

            def emit_norm(p, sub, it):
                hb = sub * D
                i0 = it * ITILE
                o_ps = o_pss.pop((p, sub, it))
                rec = oaccp.tile([1, ITILE], F32, tag="rec", name=f"r_{p}_{sub}_{it}")
                nc.vector.reciprocal(out=rec, in_=o_ps[D : D + 1, :])
                recb = oaccp.tile([D, ITILE], F32, tag="recb", name=f"rb_{p}_{sub}_{it}")
                nc.gpsimd.partition_broadcast(recb, rec)
                out_sb = outp.tile([D, ITILE], F32, tag="out_sb", name=f"ot_{p}_{sub}_{it}")
                nc.vector.tensor_mul(out=out_sb, in0=o_ps[0:D, :], in1=recb)
                nc.sync.dma_start(
                    out=o_d[p][hb : hb + D, i0 : i0 + ITILE], in_=out_sb
                )

            # software-pipelined emission, O lagged two stages: the PE stream
            # is ... S(g+1) O(g-1) S(g+2) O(g) ..., so when exp(g) releases a
            # slot, the O matmuls ahead of the refill S(g+2) in the in-order
            # PE queue have long retired and S issues immediately.
            emit_S(0)
            for i in range(len(groups)):
                emit_X(i)
                if i + 1 < len(groups):
                    emit_S(i + 1)
                if i < 4:
                    emit_pair0_batch(i)
                if i >= 1:
                    emit_O(i - 1)
            emit_O(len(groups) - 1)

    nc.compile()
    return nc


def kernel(q: np.ndarray, k: np.ndarray, v: np.ndarray) -> np.ndarray:
    if "nc" not in _CACHE:
        _CACHE["nc"] = build_bass()
    nc = _CACHE["nc"]

    in_maps = [{"qkv": pack_core_inputs(q, k, v, c)} for c in range(NCORES)]
    res = run_bass_kernel_spmd(nc, in_maps, core_ids=list(range(NCORES)))
    out = np.concatenate([res.results[c]["out"] for c in range(NCORES)], axis=0)
    return out.reshape(B, H, D, N).astype(np.float32)


def pack_core_inputs(q, k, v, c):
    """[NPAIRS, 3, 128, N] fp32: two heads stacked per pair, q/k/v interleaved."""
    qr = q.reshape(B * H, D, N)[c * HPC : (c + 1) * HPC].reshape(NPAIRS, 128, N)
    kr = k.reshape(B * H, D, N)[c * HPC : (c + 1) * HPC].reshape(NPAIRS, 128, N)
    vr = v.reshape(B * H, D, N)[c * HPC : (c + 1) * HPC].reshape(NPAIRS, 128, N)
    return np.ascontiguousarray(
        np.stack([qr, kr, vr], axis=1).astype(np.float32, copy=False)
    )


if __name__ == "__main__":
    rng = np.random.default_rng(0)
    q = rng.standard_normal((B, H, D, N), dtype=np.float32)
    k = rng.standard_normal((B, H, D, N), dtype=np.float32)
    v = rng.standard_normal((B, H, D, N), dtype=np.float32)
    out = kernel(q, k, v)
    s = np.einsum("hdi,hdj->hij", q.reshape(-1, D, N)[:2] * SCALE, k.reshape(-1, D, N)[:2])
    p = np.exp(s - s.max(-1, keepdims=True))
    p /= p.sum(-1, keepdims=True)
    ref = np.einsum("hij,hdj->hdi", p, v.reshape(-1, D, N)[:2])
    got = out.reshape(-1, D, N)[:2]
    print("rel err (2 heads):", np.linalg.norm(got - ref) / np.linalg.norm(ref))


# revision 51
# speedup vs baseline: 1.1143x; 1.0184x over previous
"""Trainium2 Bass kernel for batched multi-head attention.

Problem: q, k, v: [B=4, H=16, D=64, N=2048] fp32, layout (b, h, d, n).
    sim  = einsum('bhdi,bhdj->bhij', q * D**-0.5, k)
    attn = softmax(sim, axis=-1)
    out  = einsum('bhij,bhdj->bhdi', attn, v)

Sharding: the 64 (b, h) pairs split across 8 NeuronCores, 8 heads per core.
Each core runs an identical Bass program on its own slice; no collectives.

Per-core algorithm (flash-style, S^T layout so no P transposes are needed):
  per head pair: V^T chunks via PE transpose (both heads in one [128,128] block)
  per head, per i-tile (512 query cols):
    per group of key chunks (3,3,3,3,2,2 chunks of 128 keys):
      S^T[j, i] = K_chunk^T Q_tile     (fp32r matmuls -> 3-bank PSUM slot)
      expS = exp(0.125 * S^T)          (one ACT instr over the whole slot)
      O_acc[d|sum, i] += [V^T | 1]^T expS   (fp32r matmuls into a dedicated
                                        1-bank PSUM accumulator, all 16 chunks)
    out[d, i] = O_acc[d] * (1 / O_acc[64])  (DVE recip + DRAM-bounce broadcast)

PSUM: 2 x 3-bank S slots + 2 x 1-bank O accumulators = 8 banks.
S slots are released by ACT alone, so the steady-state cycle is
exp(g-2) -> S(g) -> exp(g): ACT (the roofline engine, 8 * 2048^2 exp elems /
128 lanes / 1.2 GHz ~= 220 us/core) runs back-to-back. Emission is
software-pipelined (S of group g+1 before O of group g). The next pair's
V^T transposes ride in the spare bank of the 2-chunk S slots (pair 0 stages
through the then-idle O accumulator banks), so pair switches cost no slot
turns.
"""

import numpy as np

import concourse.bass as bass
import concourse.mybir as mybir
import concourse.tile as tile
from concourse import bacc
from concourse.bass_utils import run_bass_kernel_spmd
from concourse.masks import make_identity

B, H, D, N = 4, 16, 64, 2048
NCORES = 8
HPC = (B * H) // NCORES  # heads per core = 8
NPAIRS = HPC // 2        # head pairs per core = 4
ITILE = 512              # query columns per i-tile (PSUM bank = 512 fp32)
NIT = N // ITILE         # 4 i-tiles
JC = 128                 # key chunk (matmul M / partition dim)
NCH = N // JC            # 16 chunks
SCALE = float(D) ** -0.5

# chunk groups per i-tile: (start, count). The trailing 2-chunk groups keep
# slot-refill cost below the preceding exp duration at i-tile boundaries.
# The very first i-tile of the kernel ramps with tiny groups so ACT starts
# as soon as the first 64 KB of K has landed.
GROUPS_STEADY = [(0, 3), (3, 3), (6, 3), (9, 3), (12, 2), (14, 2)]
GROUPS_RAMP = [(0, 1), (1, 2), (3, 3), (6, 3), (9, 3), (12, 2), (14, 2)]

F32 = mybir.dt.float32
F32R = mybir.dt.float32r

_CACHE = {}


def build_bass():
    nc = bacc.Bacc("TRN2", target_bir_lowering=False)
    qkv_h = nc.dram_tensor("qkv", [NPAIRS, 3, 128, N], F32, kind="ExternalInput")
    o_h = nc.dram_tensor("out", [HPC, D, N], F32, kind="ExternalOutput")

    qkv_d = qkv_h[:, :, :, :].rearrange("p t a n -> p a t n")  # [NPAIRS, 128, 3, N]
    o_d = o_h[:, :, :].rearrange("(p two) d n -> p (two d) n", two=2)

    with tile.TileContext(nc) as tc:
        with (
            tc.tile_pool(name="consts", bufs=1) as consts,
            tc.tile_pool(name="pairs", bufs=2) as pairs,
            tc.tile_pool(name="spsum", bufs=2, space="PSUM") as spsum,
            tc.tile_pool(name="opsum", bufs=2, space="PSUM") as opsum,
            tc.tile_pool(name="expp", bufs=4) as expp,
            tc.tile_pool(name="vtp", bufs=2) as vtp,
            tc.tile_pool(name="oaccp", bufs=2) as oaccp,
            tc.tile_pool(name="outp", bufs=3) as outp,
            tc.tile_pool(name="dramp", bufs=3, space="DRAM") as dramp,
        ):
            ident = consts.tile([128, 128], F32, tag="ident")
            make_identity(nc, ident)
            ones_dram = nc.inline_tensor(np.ones([1, 1], np.float32), name="ones_const")

            pair_ctx: list[dict | None] = [None] * NPAIRS

            def emit_pair_dma(p):
                q3 = pairs.tile([128, 3, N], F32R, tag="q3", name=f"q3_{p}")
                # split q+k from v so the S pipeline starts before v lands;
                # pair 0 additionally peels the first S group's operands so
                # the ramp isn't gated on the full 2 MB q+k transfer
                src = qkv_d[p].bitcast(F32R)
                if p == 0:
                    nc.sync.dma_start(out=q3[:, 0, 0:ITILE], in_=src[:, 0, 0:ITILE])
                    nc.sync.dma_start(out=q3[:, 1, 0:ITILE], in_=src[:, 1, 0:ITILE])
                    nc.sync.dma_start(out=q3[:, 1, ITILE:N], in_=src[:, 1, ITILE:N])
                    nc.sync.dma_start(out=q3[:, 2, 0 : N // 2], in_=src[:, 2, 0 : N // 2])
                    nc.sync.dma_start(out=q3[:, 2, N // 2 : N], in_=src[:, 2, N // 2 : N])
                    nc.sync.dma_start(out=q3[:, 0, ITILE:N], in_=src[:, 0, ITILE:N])
                else:
                    nc.sync.dma_start(out=q3[:, 0:2, :], in_=src[:, 0:2, :])
                    nc.sync.dma_start(out=q3[:, 2:3, :], in_=src[:, 2:3, :])
                vt_sb = vtp.tile([128, NCH, 2, D + 1], F32R, tag="vt", name=f"vt_{p}")
                ones_bcast = bass.AP(
                    tensor=ones_dram, offset=0, ap=[[0, 128], [0, 2 * NCH]]
                )
                nc.sync.dma_start(
                    out=vt_sb.rearrange("a b c d -> a (b c) d")[:, :, D : D + 1],
                    in_=ones_bcast.bitcast(F32R),
                )
                pair_ctx[p] = {"q3": q3, "vt": vt_sb}

            def emit_transposes(p, c0, cn, ps_region):
                """Transpose V chunks [c0, c0+cn) of pair p into ps_region
                ([128, cn*128] PSUM), then copy into vt_sb."""
                cx = pair_ctx[p]
                q3 = cx["q3"]
                for c in range(c0, c0 + cn):
                    nc.tensor.transpose(
                        out=ps_region[:, (c - c0) * 128 : (c - c0 + 1) * 128],
                        in_=q3[:, 2, c * 128 : (c + 1) * 128].bitcast(F32),
                        identity=ident,
                    )
                nc.vector.tensor_copy(
                    out=cx["vt"][:, c0 : c0 + cn, :, 0:D],
                    in_=ps_region[:, 0 : cn * 128].rearrange(
                        "a (c h d) -> a c h d", h=2, d=D
                    ),
                )

            def emit_pair0_batch(b):
                # o-slots are idle at ramp: stage pair 0's V^T through them
                vt_ps = opsum.tile([128, ITILE], F32, tag="o", name=f"vtps0_{b}")
                emit_transposes(0, 4 * b, 4, vt_ps)

            # Global chunk stream: groups of 3 chunk-units spanning i-tile,
            # head, and pair boundaries freely — every exp instruction is a
            # uniform [128, 1536] (bar the 2-unit ramp group), and every
            # slot-refill transition has maximal slack. Each unit is
            # (p, sub, it, c): one K chunk of one head's i-tile.
            units = [
                (p, sub, it, c)
                for p in range(NPAIRS)
                for sub in range(2)
                for it in range(NIT)
                for c in range(NCH)
            ]
            groups = [units[0:2]]
            groups += [units[i : i + 3] for i in range(2, len(units), 3)]
            slots: dict[int, bass.AP] = {}
            exs: dict[int, bass.AP] = {}
            o_pss: dict[tuple, bass.AP] = {}

            def emit_S(i):
                s_slot = spsum.tile([128, 3 * ITILE], F32, tag="s", name=f"s_{i}")
                for kk, (p, sub, it, c) in enumerate(groups[i]):
                    if sub == 0 and it == 0 and c == 0:
                        if p == 0:
                            emit_pair_dma(0)
                        if p + 1 < NPAIRS:
                            emit_pair_dma(p + 1)
                    q3 = pair_ctx[p]["q3"]
                    hb = sub * D
                    nc.tensor.matmul(
                        out=s_slot[:, kk * ITILE : (kk + 1) * ITILE],
                        lhsT=q3[hb : hb + D, 1, c * JC : (c + 1) * JC],
                        rhs=q3[hb : hb + D, 0, it * ITILE : (it + 1) * ITILE],
                        start=True,
                        stop=True,
                    )
                slots[i] = s_slot

            def emit_X(i):
                cn = len(groups[i])
                ex = expp.tile([128, cn * ITILE], F32R, tag="exp", name=f"x_{i}")
                nc.scalar.activation(
                    out=ex,
                    in_=slots[i][:, 0 : cn * ITILE],
                    func=mybir.ActivationFunctionType.Exp,
                    scale=SCALE,
                )
                exs[i] = ex

            def emit_O(i):
                s_slot, ex = slots.pop(i), exs.pop(i)
                for kk, (p, sub, it, c) in enumerate(groups[i]):
                    cx = pair_ctx[p]
                    if c == 0:
                        o_ps = opsum.tile(
                            [128, ITILE], F32, tag="o", name=f"ops_{i}_{kk}"
                        )
                        o_pss[(p, sub, it)] = o_ps
                    else:
                        o_ps = o_pss[(p, sub, it)]
                    nc.tensor.matmul(
                        out=o_ps[0 : D + 1, :],
                        lhsT=cx["vt"][:, c, sub, :],
                        rhs=ex[:, kk * ITILE : (kk + 1) * ITILE],
                        start=(c == 0),
                        stop=(c == NCH - 1),
                    )
                    # second head's i-tiles stage 4 chunks of pair p+1's V^T
                    # through the o-slot idling between norm(it-1) release
                    # and accum(it+1) alloc (c==6 leaves the preceding norm
                    # ~2 us to drain so the borrow never stalls the PE queue)
                    if c == 6 and sub == 1 and p + 1 < NPAIRS:
                        vt_ps = opsum.tile(
                            [128, ITILE], F32, tag="o", name=f"vtr_{i}"
                        )
                        emit_transposes(p + 1, 4 * it, 4, vt_ps)
                    if c == NCH - 1:
                        emit_norm(p, sub, it)

            def emit_norm(p, sub, it):
                hb = sub * D
                i0 = it * ITILE
                o_ps = o_pss.pop((p, sub, it))
                rec = oaccp.tile([1, ITILE], F32, tag="rec", name=f"r_{p}_{sub}_{it}")
                nc.vector.reciprocal(out=rec, in_=o_ps[D : D + 1, :])
                recb = oaccp.tile([D, ITILE], F32, tag="recb", name=f"rb_{p}_{sub}_{it}")
                nc.gpsimd.partition_broadcast(recb, rec)
                out_sb = outp.tile([D, ITILE], F32, tag="out_sb", name=f"ot_{p}_{sub}_{it}")
                nc.vector.tensor_mul(out=out_sb, in0=o_ps[0:D, :], in1=recb)
                nc.sync.dma_start(
                    out=o_d[p][hb : hb + D, i0 : i0 + ITILE], in_=out_sb
                )

            # software-pipelined emission, O lagged two stages: the PE stream
            # is ... S(g+1) O(g-1) S(g+2) O(g) ..., so when exp(g) releases a
            # slot, the O matmuls ahead of the refill S(g+2) in the in-order
            # PE queue have long retired and S issues immediately.
            emit_S(0)
            for i in range(len(groups)):
                emit_X(i)
                if i + 1 < len(groups):
                    emit_S(i + 1)
                if i < 4:
                    emit_pair0_batch(i)
                if i >= 1:
                    emit_O(i - 1)
            emit_O(len(groups) - 1)

    nc.compile()
    return nc


def kernel(q: np.ndarray, k: np.ndarray, v: np.ndarray) -> np.ndarray:
    if "nc" not in _CACHE:
        _CACHE["nc"] = build_bass()
    nc = _CACHE["nc"]

    in_maps = [{"qkv": pack_core_inputs(q, k, v, c)} for c in range(NCORES)]
    res = run_bass_kernel_spmd(nc, in_maps, core_ids=list(range(NCORES)))
    out = np.concatenate([res.results[c]["out"] for c in range(NCORES)], axis=0)
    return out.reshape(B, H, D, N).astype(np.float32)


def pack_core_inputs(q, k, v, c):
    """[NPAIRS, 3, 128, N] fp32: two heads stacked per pair, q/k/v interleaved."""
    qr = q.reshape(B * H, D, N)[c * HPC : (c + 1) * HPC].reshape(NPAIRS, 128, N)
    kr = k.reshape(B * H, D, N)[c * HPC : (c + 1) * HPC].reshape(NPAIRS, 128, N)
    vr = v.reshape(B * H, D, N)[c * HPC : (c + 1) * HPC].reshape(NPAIRS, 128, N)
    return np.ascontiguousarray(
        np.stack([qr, kr, vr], axis=1).astype(np.float32, copy=False)
    )


if __name__ == "__main__":
    rng = np.random.default_rng(0)
    q = rng.standard_normal((B, H, D, N), dtype=np.float32)
    k = rng.standard_normal((B, H, D, N), dtype=np.float32)
    v = rng.standard_normal((B, H, D, N), dtype=np.float32)
    out = kernel(q, k, v)
    s = np.einsum("hdi,hdj->hij", q.reshape(-1, D, N)[:2] * SCALE, k.reshape(-1, D, N)[:2])
    p = np.exp(s - s.max(-1, keepdims=True))
    p /= p.sum(-1, keepdims=True)
    ref = np.einsum("hij,hdj->hdi", p, v.reshape(-1, D, N)[:2])
    got = out.reshape(-1, D, N)[:2]
    print("rel err (2 heads):", np.linalg.norm(got - ref) / np.linalg.norm(ref))


# revision 52
# speedup vs baseline: 1.1163x; 1.0018x over previous
"""Trainium2 Bass kernel for batched multi-head attention.

Problem: q, k, v: [B=4, H=16, D=64, N=2048] fp32, layout (b, h, d, n).
    sim  = einsum('bhdi,bhdj->bhij', q * D**-0.5, k)
    attn = softmax(sim, axis=-1)
    out  = einsum('bhij,bhdj->bhdi', attn, v)

Sharding: the 64 (b, h) pairs split across 8 NeuronCores, 8 heads per core.
Each core runs an identical Bass program on its own slice; no collectives.

Per-core algorithm (flash-style, S^T layout so no P transposes are needed):
  per head pair: V^T chunks via PE transpose (both heads in one [128,128] block)
  per head, per i-tile (512 query cols):
    per group of key chunks (3,3,3,3,2,2 chunks of 128 keys):
      S^T[j, i] = K_chunk^T Q_tile     (fp32r matmuls -> 3-bank PSUM slot)
      expS = exp(0.125 * S^T)          (one ACT instr over the whole slot)
      O_acc[d|sum, i] += [V^T | 1]^T expS   (fp32r matmuls into a dedicated
                                        1-bank PSUM accumulator, all 16 chunks)
    out[d, i] = O_acc[d] * (1 / O_acc[64])  (DVE recip + DRAM-bounce broadcast)

PSUM: 2 x 3-bank S slots + 2 x 1-bank O accumulators = 8 banks.
S slots are released by ACT alone, so the steady-state cycle is
exp(g-2) -> S(g) -> exp(g): ACT (the roofline engine, 8 * 2048^2 exp elems /
128 lanes / 1.2 GHz ~= 220 us/core) runs back-to-back. Emission is
software-pipelined (S of group g+1 before O of group g). The next pair's
V^T transposes ride in the spare bank of the 2-chunk S slots (pair 0 stages
through the then-idle O accumulator banks), so pair switches cost no slot
turns.
"""

import numpy as np

import concourse.bass as bass
import concourse.mybir as mybir
import concourse.tile as tile
from concourse import bacc
from concourse.bass_utils import run_bass_kernel_spmd
from concourse.masks import make_identity

B, H, D, N = 4, 16, 64, 2048
NCORES = 8
HPC = (B * H) // NCORES  # heads per core = 8
NPAIRS = HPC // 2        # head pairs per core = 4
ITILE = 512              # query columns per i-tile (PSUM bank = 512 fp32)
NIT = N // ITILE         # 4 i-tiles
JC = 128                 # key chunk (matmul M / partition dim)
NCH = N // JC            # 16 chunks
SCALE = float(D) ** -0.5

# chunk groups per i-tile: (start, count). The trailing 2-chunk groups keep
# slot-refill cost below the preceding exp duration at i-tile boundaries.
# The very first i-tile of the kernel ramps with tiny groups so ACT starts
# as soon as the first 64 KB of K has landed.
GROUPS_STEADY = [(0, 3), (3, 3), (6, 3), (9, 3), (12, 2), (14, 2)]
GROUPS_RAMP = [(0, 1), (1, 2), (3, 3), (6, 3), (9, 3), (12, 2), (14, 2)]

F32 = mybir.dt.float32
F32R = mybir.dt.float32r

_CACHE = {}


def build_bass():
    nc = bacc.Bacc("TRN2", target_bir_lowering=False)
    qkv_h = nc.dram_tensor("qkv", [NPAIRS, 3, 128, N], F32, kind="ExternalInput")
    o_h = nc.dram_tensor("out", [HPC, D, N], F32, kind="ExternalOutput")

    qkv_d = qkv_h[:, :, :, :].rearrange("p t a n -> p a t n")  # [NPAIRS, 128, 3, N]
    o_d = o_h[:, :, :].rearrange("(p two) d n -> p (two d) n", two=2)

    with tile.TileContext(nc) as tc:
        with (
            tc.tile_pool(name="consts", bufs=1) as consts,
            tc.tile_pool(name="pairs", bufs=2) as pairs,
            tc.tile_pool(name="spsum", bufs=2, space="PSUM") as spsum,
            tc.tile_pool(name="opsum", bufs=2, space="PSUM") as opsum,
            tc.tile_pool(name="expp", bufs=4) as expp,
            tc.tile_pool(name="vtp", bufs=2) as vtp,
            tc.tile_pool(name="oaccp", bufs=2) as oaccp,
            tc.tile_pool(name="outp", bufs=3) as outp,
            tc.tile_pool(name="dramp", bufs=3, space="DRAM") as dramp,
        ):
            ident = consts.tile([128, 128], F32, tag="ident")
            make_identity(nc, ident)
            ones_dram = nc.inline_tensor(np.ones([1, 1], np.float32), name="ones_const")

            pair_ctx: list[dict | None] = [None] * NPAIRS

            def emit_pair_dma(p):
                q3 = pairs.tile([128, 3, N], F32R, tag="q3", name=f"q3_{p}")
                # split q+k from v so the S pipeline starts before v lands;
                # pair 0 additionally peels the first S group's operands so
                # the ramp isn't gated on the full 2 MB q+k transfer
                src = qkv_d[p].bitcast(F32R)
                if p == 0:
                    nc.sync.dma_start(out=q3[:, 0, 0:ITILE], in_=src[:, 0, 0:ITILE])
                    nc.sync.dma_start(out=q3[:, 1, 0:ITILE], in_=src[:, 1, 0:ITILE])
                    nc.sync.dma_start(out=q3[:, 1, ITILE:N], in_=src[:, 1, ITILE:N])
                    nc.sync.dma_start(out=q3[:, 2, 0 : N // 2], in_=src[:, 2, 0 : N // 2])
                    nc.sync.dma_start(out=q3[:, 2, N // 2 : N], in_=src[:, 2, N // 2 : N])
                    nc.sync.dma_start(out=q3[:, 0, ITILE:N], in_=src[:, 0, ITILE:N])
                else:
                    nc.sync.dma_start(out=q3[:, 0:2, :], in_=src[:, 0:2, :])
                    nc.sync.dma_start(out=q3[:, 2:3, :], in_=src[:, 2:3, :])
                vt_sb = vtp.tile([128, NCH, 2, D + 1], F32R, tag="vt", name=f"vt_{p}")
                ones_bcast = bass.AP(
                    tensor=ones_dram, offset=0, ap=[[0, 128], [0, 2 * NCH]]
                )
                nc.sync.dma_start(
                    out=vt_sb.rearrange("a b c d -> a (b c) d")[:, :, D : D + 1],
                    in_=ones_bcast.bitcast(F32R),
                )
                pair_ctx[p] = {"q3": q3, "vt": vt_sb}

            def emit_transposes(p, c0, cn, ps_region):
                """Transpose V chunks [c0, c0+cn) of pair p into ps_region
                ([128, cn*128] PSUM), then copy into vt_sb."""
                cx = pair_ctx[p]
                q3 = cx["q3"]
                for c in range(c0, c0 + cn):
                    nc.tensor.transpose(
                        out=ps_region[:, (c - c0) * 128 : (c - c0 + 1) * 128],
                        in_=q3[:, 2, c * 128 : (c + 1) * 128].bitcast(F32),
                        identity=ident,
                    )
                nc.vector.tensor_copy(
                    out=cx["vt"][:, c0 : c0 + cn, :, 0:D],
                    in_=ps_region[:, 0 : cn * 128].rearrange(
                        "a (c h d) -> a c h d", h=2, d=D
                    ),
                )

            def emit_pair0_batch(b):
                # o-slots are idle at ramp: stage pair 0's V^T through them
                vt_ps = opsum.tile([128, ITILE], F32, tag="o", name=f"vtps0_{b}")
                emit_transposes(0, 4 * b, 4, vt_ps)

            # Global chunk stream: groups of 3 chunk-units spanning i-tile,
            # head, and pair boundaries freely — every exp instruction is a
            # uniform [128, 1536] (bar the 2-unit ramp group), and every
            # slot-refill transition has maximal slack. Each unit is
            # (p, sub, it, c): one K chunk of one head's i-tile.
            units = [
                (p, sub, it, c)
                for p in range(NPAIRS)
                for sub in range(2)
                for it in range(NIT)
                for c in range(NCH)
            ]
            groups = [units[0:2]]
            groups += [units[i : i + 3] for i in range(2, len(units), 3)]
            slots: dict[int, bass.AP] = {}
            exs: dict[int, bass.AP] = {}
            o_pss: dict[tuple, bass.AP] = {}

            def emit_S(i):
                s_slot = spsum.tile([128, 3 * ITILE], F32, tag="s", name=f"s_{i}")
                for kk, (p, sub, it, c) in enumerate(groups[i]):
                    if sub == 0 and it == 0 and c == 0:
                        if p == 0:
                            emit_pair_dma(0)
                        if p + 1 < NPAIRS:
                            emit_pair_dma(p + 1)
                    q3 = pair_ctx[p]["q3"]
                    hb = sub * D
                    nc.tensor.matmul(
                        out=s_slot[:, kk * ITILE : (kk + 1) * ITILE],
                        lhsT=q3[hb : hb + D, 1, c * JC : (c + 1) * JC],
                        rhs=q3[hb : hb + D, 0, it * ITILE : (it + 1) * ITILE],
                        start=True,
                        stop=True,
                    )
                slots[i] = s_slot

            def emit_X(i):
                cn = len(groups[i])
                ex = expp.tile([128, cn * ITILE], F32R, tag="exp", name=f"x_{i}")
                nc.scalar.activation(
                    out=ex,
                    in_=slots[i][:, 0 : cn * ITILE],
                    func=mybir.ActivationFunctionType.Exp,
                    scale=SCALE,
                )
                exs[i] = ex

            def emit_O(i):
                s_slot, ex = slots.pop(i), exs.pop(i)
                for kk, (p, sub, it, c) in enumerate(groups[i]):
                    cx = pair_ctx[p]
                    if c == 0:
                        o_ps = opsum.tile(
                            [128, ITILE], F32, tag="o", name=f"ops_{i}_{kk}"
                        )
                        o_pss[(p, sub, it)] = o_ps
                    else:
                        o_ps = o_pss[(p, sub, it)]
                    nc.tensor.matmul(
                        out=o_ps[0 : D + 1, :],
                        lhsT=cx["vt"][:, c, sub, :],
                        rhs=ex[:, kk * ITILE : (kk + 1) * ITILE],
                        start=(c == 0),
                        stop=(c == NCH - 1),
                    )
                    # second head's i-tiles stage 4 chunks of pair p+1's V^T
                    # through the o-slot idling between norm(it-1) release
                    # and accum(it+1) alloc (c==6 leaves the preceding norm
                    # ~2 us to drain so the borrow never stalls the PE queue)
                    if c == 6 and sub == 1 and p + 1 < NPAIRS:
                        vt_ps = opsum.tile(
                            [128, ITILE], F32, tag="o", name=f"vtr_{i}"
                        )
                        emit_transposes(p + 1, 4 * it, 4, vt_ps)
                    if c == NCH - 1:
                        emit_norm(p, sub, it)

            def emit_norm(p, sub, it):
                hb = sub * D
                i0 = it * ITILE
                o_ps = o_pss.pop((p, sub, it))
                rec = oaccp.tile([1, ITILE], F32, tag="rec", name=f"r_{p}_{sub}_{it}")
                nc.vector.reciprocal(out=rec, in_=o_ps[D : D + 1, :])
                recb = oaccp.tile([D, ITILE], F32, tag="recb", name=f"rb_{p}_{sub}_{it}")
                nc.gpsimd.partition_broadcast(recb, rec)
                out_sb = outp.tile([D, ITILE], F32, tag="out_sb", name=f"ot_{p}_{sub}_{it}")
                nc.vector.tensor_mul(out=out_sb, in0=o_ps[0:D, :], in1=recb)
                nc.sync.dma_start(
                    out=o_d[p][hb : hb + D, i0 : i0 + ITILE], in_=out_sb
                )

            # software-pipelined emission, O lagged two stages: the PE stream
            # is ... S(g+1) O(g-1) S(g+2) O(g) ..., so when exp(g) releases a
            # slot, the O matmuls ahead of the refill S(g+2) in the in-order
            # PE queue have long retired and S issues immediately.
            emit_S(0)
            for i in range(len(groups)):
                emit_X(i)
                if i + 1 < len(groups):
                    emit_S(i + 1)
                if i < 2:
                    # two batches per step: b0/b1 claim both idle o-slots in
                    # parallel before the first O accumulator takes one
                    emit_pair0_batch(2 * i)
                    emit_pair0_batch(2 * i + 1)
                if i >= 1:
                    emit_O(i - 1)
            emit_O(len(groups) - 1)

    nc.compile()
    return nc


def kernel(q: np.ndarray, k: np.ndarray, v: np.ndarray) -> np.ndarray:
    if "nc" not in _CACHE:
        _CACHE["nc"] = build_bass()
    nc = _CACHE["nc"]

    in_maps = [{"qkv": pack_core_inputs(q, k, v, c)} for c in range(NCORES)]
    res = run_bass_kernel_spmd(nc, in_maps, core_ids=list(range(NCORES)))
    out = np.concatenate([res.results[c]["out"] for c in range(NCORES)], axis=0)
    return out.reshape(B, H, D, N).astype(np.float32)


def pack_core_inputs(q, k, v, c):
    """[NPAIRS, 3, 128, N] fp32: two heads stacked per pair, q/k/v interleaved."""
    qr = q.reshape(B * H, D, N)[c * HPC : (c + 1) * HPC].reshape(NPAIRS, 128, N)
    kr = k.reshape(B * H, D, N)[c * HPC : (c + 1) * HPC].reshape(NPAIRS, 128, N)
    vr = v.reshape(B * H, D, N)[c * HPC : (c + 1) * HPC].reshape(NPAIRS, 128, N)
    return np.ascontiguousarray(
        np.stack([qr, kr, vr], axis=1).astype(np.float32, copy=False)
    )


if __name__ == "__main__":
    rng = np.random.default_rng(0)
    q = rng.standard_normal((B, H, D, N), dtype=np.float32)
    k = rng.standard_normal((B, H, D, N), dtype=np.float32)
    v = rng.standard_normal((B, H, D, N), dtype=np.float32)
    out = kernel(q, k, v)
    s = np.einsum("hdi,hdj->hij", q.reshape(-1, D, N)[:2] * SCALE, k.reshape(-1, D, N)[:2])
    p = np.exp(s - s.max(-1, keepdims=True))
    p /= p.sum(-1, keepdims=True)
    ref = np.einsum("hij,hdj->hdi", p, v.reshape(-1, D, N)[:2])
    got = out.reshape(-1, D, N)[:2]
    print("rel err (2 heads):", np.linalg.norm(got - ref) / np.linalg.norm(ref))
